# revision 10
# baseline (speedup 1.0000x reference)
"""Trainium2 Bass kernel for nn_EncoderLayer_39857296507465 (Performer encoder layer).

Sharding: 8-way over the flattened (B*S)=16384 token axis -> 2048 tokens/core
(half of one batch element per core). Weights replicated. The only cross-core
communication is the per-(batch,head) kv/ksum reduction over the sequence: a
532 KB AllReduce between core pairs {0,1},{2,3},{4,5},{6,7}.

Host I/O is optimized for the axon tunnel (~45 MB/s, ~85 ms/dispatch):
  - x is staged in its natural [B*S, D] f32 layout (zero host preprocessing,
    one 67 MB transfer, crc-cached across calls); the kernel transposes it
    to feature-major and casts to bf16 on device (PE transposes).
  - weights are prepped on host (~25 MB), transferred once to core 0, then
    replicated across the 8 cores with an on-device reshard
    (device_put -> NamedSharding(mesh, P())); cached across calls.
  - the output is written token-major [T, D] f16 on device, so the host
    gather is a single 33 MB pull + astype(float32).reshape - no transpose.

Device layout: activations feature-major (xT = [D, tokens]); every linear layer
is matmul(lhsT=native-weight-chunk, rhs=featT). Exceptions: v and phi(k) are
token-major (tokens contract in the kv einsum); attention outputs are computed
token-major so the 1/(pq.ksum+eps) normalizer is a per-partition tensor_scalar,
then a PE-transpose pass re-merges heads feature-major for the Wo projection.

SBUF is tight, so q/k/v and the LN residuals stream through DRAM scratch:
  qkd  [2048, T] bf16   q rows 0:1024, k rows 1024:2048
  vad  [SC, 128, H*65]  v_aug token-major chunks (65th col of each head = 1)
  qhld [16, 2, T] bf16  hi/lo rows of -|q|^2/2 (+ln(1/sqrt M)) per head
  xtfd [D, T] f32       feature-major f32 x (residual input)
  r1d/r2d [D, T] f32    LN residual inputs
Per-token row broadcasts run on the PE (K=1 f32 / K=2 bf16 hi+lo matmuls with
all-ones stationary operands, accumulated into the consumer's PSUM tile). The
phi(k) -|k|^2/2 bias is applied token-major instead: ebias = exp(c - sq_k) is
per-partition there, so pk = exp(proj)*ebias + eps via one dual tensor_scalar.
"""
import os
import sys
import zlib
sys.path.insert(0, '/opt/trn_rl_repo')

import numpy as np
import ml_dtypes

import concourse.bass as bass
from concourse import bacc
import concourse.mybir as mybir
import concourse.tile as tile
from concourse.masks import make_identity
from concourse.bass_utils import run_bass_kernel_spmd  # noqa: F401 (doc pointer)

F32 = mybir.dt.float32
BF16 = mybir.dt.bfloat16
F16 = mybir.dt.float16
AF = mybir.ActivationFunctionType
OP = mybir.AluOpType

B, S, D, H, M, DFF = 4, 4096, 1024, 16, 128, 4096
DH = D // H                      # 64
LN_EPS = 1e-6
KERN_EPS = 1e-6
NCORES = 8
T = (B * S) // NCORES            # 2048 tokens/core
SC = T // 128                    # 16 token chunks
KC = D // 128                    # 8 feature chunks
FC = DFF // 128                  # 32 dff chunks
NT = T // 512                    # 4 moving tiles
VW = DH + 1                      # 65 (v | ones)
VJ = 4 * VW                      # 260: quarter of the v_aug row
KVP = 1280                       # kv store padded to PSUM-bank-aligned groups
CLNM = -0.5 * float(np.log(M))   # folds 1/sqrt(M) into the exp bias
KPHASES = int(os.environ.get("KPHASES", "9"))  # debug: truncate after phase N
KP3 = int(os.environ.get("KP3", "4"))  # phase-3 sub-bisect: 1=dma 2=+proj 3=+ebias 4=full


HQUAD = [0, 2, 4, 6, 8, 10, 12, 14, 1, 3, 5, 7, 9, 11, 13, 15]
HSLOT = {h: i for i, h in enumerate(HQUAD)}

WEIGHT_NAMES = ["wqkp", "wvpj", "bqk", "bva", "omd", "ehalf", "wop", "bo",
                "w1p", "b1", "w2p", "b2", "g1", "be1", "g2", "be2"]
RAW_WEIGHT_KEYS = ["Wq", "bq", "Wk", "bk", "Wv", "bv", "Wo", "bo", "omega",
                   "W1", "b1", "W2", "b2", "g1", "beta1", "g2", "beta2"]


def kvoff(h):
    """col offset of head h inside the [128, KVP] kv store (bank-aligned)."""
    return 512 * (h // 7) + VW * (h % 7)

_CACHE = {}


def _build():
    nc = bacc.Bacc(None, num_devices=NCORES)

    io = {}
    def inp(name, shape, dt):
        io[name] = nc.dram_tensor(name, shape, dt, kind="ExternalInput")
    inp("xin", [T, D], F32)
    inp("wqkp", [16, 128, KC * 128], BF16)
    inp("wvpj", [4, 128, KC * VJ], BF16)
    inp("bqk", [128, 16], F32)
    inp("bva", [1, H * VW], BF16)
    inp("omd", [128, M], BF16)
    inp("ehalf", [128, KC * H], BF16)
    inp("wop", [KC, 128, KC * 128], BF16)
    inp("bo", [128, KC], F32)
    inp("w1p", [FC, 128, KC * 128], BF16)
    inp("b1", [128, FC], F32)
    inp("w2p", [FC, 128, KC * 128], BF16)
    inp("b2", [128, KC], F32)
    inp("g1", [128, KC], F32)
    inp("be1", [128, KC], F32)
    inp("g2", [128, KC], F32)
    inp("be2", [128, KC], F32)
    io["out"] = nc.dram_tensor("out", [T, D], F16, kind="ExternalOutput")

    with tile.TileContext(nc) as tc:
        _emit(nc, tc, io)
    nc.finalize()
    return nc


def _emit(nc, tc, io):
    from contextlib import ExitStack
    ctx = ExitStack()
    with ctx:
        const = ctx.enter_context(tc.tile_pool(name="const", bufs=1))
        acts = ctx.enter_context(tc.tile_pool(name="acts", bufs=1))
        wstr = ctx.enter_context(tc.tile_pool(name="wstr", bufs=4))
        sbf = ctx.enter_context(tc.tile_pool(name="sbf", bufs=7))
        sf32 = ctx.enter_context(tc.tile_pool(name="sf32", bufs=3))
        rows = ctx.enter_context(tc.tile_pool(name="rows", bufs=1))
        ps = ctx.enter_context(tc.tile_pool(name="ps", bufs=4, space="PSUM"))
        ps1 = ctx.enter_context(tc.tile_pool(name="ps1", bufs=1, space="PSUM"))
        dram = ctx.enter_context(tc.tile_pool(name="dram", bufs=1, space="DRAM"))

        def SB(tag, shape=(128, T), dt=BF16):
            return sbf.tile(list(shape), dt, name=tag, tag="sbf")

        def PP(tag, shape=(128, 512), dt=F32):
            return ps.tile(list(shape), dt, name=tag, tag="pp",
                           padded_shape=[128, 512 if dt == F32 else 1024])

        def ACC4(tag, shape=(128, 2048)):
            return ps1.tile(list(shape), F32, name=tag, tag="acc4",
                            padded_shape=[128, 2048])

        def SF(tag, shape=(128, T)):
            return sf32.tile(list(shape), F32, name=tag, tag="sf32")

        # ---------------- constants ----------------
        allones = const.tile([128, 128], BF16, name="allones")
        nc.vector.memset(allones[:], 1.0)
        ones1f = const.tile([1, 128], F32, name="ones1f")
        nc.vector.memset(ones1f[:], 1.0)
        identb = const.tile([128, 128], BF16, name="identb")
        make_identity(nc, identb[:])
        identf = const.tile([128, 128], F32, name="identf")
        make_identity(nc, identf[:])

        def cin(name, shape, dt):
            t = const.tile(shape, dt, name=name, tag=name)
            nc.sync.dma_start(t[:], io[name][:])
            return t
        bqk_t = cin("bqk", [128, 16], F32)
        bva_t = cin("bva", [1, H * VW], BF16)
        omd_t = cin("omd", [128, M], BF16)
        eh_t = cin("ehalf", [128, KC * H], BF16)
        bo_t = cin("bo", [128, KC], F32)
        b1_t = cin("b1", [128, FC], F32)
        b2_t = cin("b2", [128, KC], F32)
        g1_t = cin("g1", [128, KC], F32)
        be1_t = cin("be1", [128, KC], F32)
        g2_t = cin("g2", [128, KC], F32)
        be2_t = cin("be2", [128, KC], F32)

        # ---------------- DRAM scratch ----------------
        qkd = dram.tile([2048, T], BF16, name="qkd")
        vad = dram.tile([SC, 128, H * VW], BF16, name="vad")
        qhld = dram.tile([16, 2, T], BF16, name="qhld")
        xtfd = dram.tile([D, T], F32, name="xtfd")
        r1d = dram.tile([D, T], F32, name="r1d")
        r2d = dram.tile([D, T], F32, name="r2d")

        def _finish_early():
            mark = SB("mark", (128, 512), F16)
            nc.vector.memset(mark[:], 1.0)
            nc.sync.dma_start(io["out"][0:128, 0:512], mark[:])

        # ============ Phase 0: transpose x on device ============
        # xin [T, D] f32 token-major -> xt (bf16, feature-major, resident)
        #                             + xtfd (f32, feature-major, DRAM)
        xt = []
        for k in range(KC):
            t = acts.tile([128, T], BF16, name=f"xt{k}", tag=f"xt{k}")
            xt.append(t)
        for sc in range(SC):
            xrow = SF("xrow", (128, D))
            nc.sync.dma_start(xrow[:], io["xin"][128 * sc:128 * (sc + 1), :])
            xfb = SF("xfb", (128, D))
            for oc in range(KC):
                ptx = PP("ptx", (128, 128))
                nc.tensor.transpose(ptx[:], xrow[:, 128 * oc:128 * (oc + 1)],
                                    identf[:])
                nc.vector.tensor_copy(xt[oc][:, 128 * sc:128 * (sc + 1)],
                                      ptx[:])
                nc.scalar.activation(xfb[:, 128 * oc:128 * (oc + 1)], ptx[:],
                                     AF.Identity)
            nc.sync.dma_start(
                xtfd[:, 128 * sc:128 * (sc + 1)].rearrange(
                    "(o p) t -> p o t", o=KC),
                xfb[:].rearrange("p (o t) -> p o t", o=KC))

        # ============ Phase 1a: qT, kT -> qkd ============
        for oc in range(16):
            wtile = wstr.tile([128, KC * 128], BF16, name="wstrip", tag="wstrip")
            nc.scalar.dma_start(wtile[:], io["wqkp"][oc])
            stage = SB("qkstage")
            for nt in range(NT):
                pqk = PP("pqk")
                for k in range(KC):
                    nc.tensor.matmul(pqk[:], wtile[:, 128 * k:128 * (k + 1)],
                                     xt[k][:, 512 * nt:512 * (nt + 1)],
                                     start=(k == 0), stop=(k == KC - 1))
                nc.scalar.activation(stage[:, 512 * nt:512 * (nt + 1)], pqk[:],
                                     AF.Identity, bias=bqk_t[:, oc:oc + 1])
            nc.sync.dma_start(qkd[128 * oc:128 * (oc + 1), :], stage[:])

        # ============ Phase 1b: v_aug -> vad ============
        for j in range(4):
            wvj = wstr.tile([128, KC * VJ], BF16, name="wvj", tag="wvj")
            nc.scalar.dma_start(wvj[:], io["wvpj"][j])
            for sc in range(SC):
                pv = PP("pv", (128, VJ))
                for k in range(KC):
                    nc.tensor.matmul(pv[:], xt[k][:, 128 * sc:128 * (sc + 1)],
                                     wvj[:, VJ * k:VJ * (k + 1)],
                                     start=(k == 0), stop=False)
                nc.tensor.matmul(pv[:], allones[0:1, :],
                                 bva_t[:, VJ * j:VJ * (j + 1)],
                                 start=False, stop=True)
                if sc % 4 == 0:
                    vstage = SB("vstage", (128, 4 * VJ))
                nc.vector.tensor_copy(
                    vstage[:, VJ * (sc % 4):VJ * (sc % 4 + 1)], pv[:])
                if sc % 4 == 3:
                    nc.sync.dma_start(
                        vad[sc - 3:sc + 1][:, :, VJ * j:VJ * (j + 1)].rearrange(
                            "s p c -> p s c"),
                        vstage[:].rearrange("p (s c) -> p s c", s=4))

        if KPHASES < 2:
            _finish_early()
            return
        # ============ Phase 2: -|q|^2/2 and -|k|^2/2 ============
        ebias = rows.tile([128, SC * H], F32, name="ebias", tag="ebias")
        for which in ("q", "k"):
            psq = ACC4("sqps", (16, T))
            for k in range(KC):
                r0 = (0 if which == "q" else 1024) + 128 * k
                src = SB("sqsrc")
                nc.sync.dma_start(src[:], qkd[r0:r0 + 128, :])
                sqr = SB("sqr")
                nc.vector.tensor_tensor(out=sqr[:], in0=src[:], in1=src[:],
                                        op=OP.mult)
                for nt in range(NT):
                    nc.tensor.matmul(psq[:, 512 * nt:512 * (nt + 1)],
                                     eh_t[:, 16 * k:16 * (k + 1)],
                                     sqr[:, 512 * nt:512 * (nt + 1)],
                                     start=(k == 0), stop=(k == KC - 1))
            if which == "q":
                hi = SB("hiq", (16, T))
                nc.vector.tensor_scalar_add(hi[:], psq[:], CLNM)
                lo = SB("loq", (16, T))
                nc.vector.scalar_tensor_tensor(out=lo[:], in0=psq[:], scalar=CLNM,
                                               in1=hi[:], op0=OP.add,
                                               op1=OP.subtract)
                nc.sync.dma_start(qhld[:, 0, :], hi[:])
                nc.sync.dma_start(qhld[:, 1, :], lo[:])
            else:
                # k-side: token-major ebias = exp(CLNM - sq_k)
                sqk = SF("sqkrows", (16, T))
                nc.vector.tensor_scalar_add(sqk[:], psq[:], CLNM)
                for sc in range(SC):
                    ptr = PP("ptre", (128, 16))
                    nc.tensor.transpose(ptr[:], sqk[:, 128 * sc:128 * (sc + 1)],
                                        identf[0:16, 0:16])
                    nc.scalar.activation(ebias[:, 16 * sc:16 * (sc + 1)], ptr[:],
                                         AF.Exp)

        if KPHASES < 3:
            _finish_early()
            return
        # ============ Phase 3: pk (token-major) + kv ============
        kv_all = rows.tile([128, KVP], F32, name="kv_all", tag="kv_all")
        nc.vector.memset(kv_all[:], 0.0)  # pad cols stay 0 (read by DMA/copy)
        for sc in range(SC):
            kcs = SB("kcs", (128, KC * 128))
            nc.sync.dma_start(
                kcs[:].rearrange("p (j t) -> p j t", j=KC),
                qkd[1024:2048, 128 * sc:128 * (sc + 1)].rearrange(
                    "(j p) t -> p j t", j=KC))
            vts = SB("vts", (128, H * VW))
            nc.sync.dma_start(vts[:], vad[sc])
            pk = SB("pk")
            if KP3 < 2:
                # consume the DMAs so they aren't dead-code eliminated
                nc.vector.tensor_copy(pk[:, 0:KC * 128], kcs[:])
                nc.vector.tensor_copy(pk[:, 1024:1536], vts[:, 0:512])
                nc.vector.tensor_tensor(out=kv_all[:, 0:512], in0=pk[:, 0:512],
                                        in1=kv_all[:, 0:512], op=OP.add)
                continue
            for q4 in range(4):
                # all heads in a quad share the same input base partition so
                # one PSUM bank never mixes base-0 and base-64 matmuls (HW bug)
                ppk = PP("ppk")
                for hh in range(4):
                    h = HQUAD[4 * q4 + hh]
                    base = 64 * (h % 2)
                    nc.tensor.matmul(ppk[:, 128 * hh:128 * (hh + 1)],
                                     kcs[base:base + 64,
                                         128 * (h // 2):128 * (h // 2) + 128],
                                     omd_t[base:base + 64, :],
                                     start=True, stop=True)
                if KP3 == 2:
                    nc.vector.tensor_copy(pk[:, 512 * q4:512 * (q4 + 1)], ppk[:])
                else:
                    nc.scalar.activation(pk[:, 512 * q4:512 * (q4 + 1)], ppk[:],
                                         AF.Exp)
            if KP3 >= 3:
                for h in range(H):
                    hs = HSLOT[h]
                    nc.vector.tensor_scalar(out=pk[:, 128 * hs:128 * (hs + 1)],
                                            in0=pk[:, 128 * hs:128 * (hs + 1)],
                                            scalar1=ebias[:, 16 * sc + h:16 * sc + h + 1],
                                            scalar2=KERN_EPS,
                                            op0=OP.mult, op1=OP.add)
            if KP3 < 4:
                nc.vector.tensor_tensor(out=kv_all[:, 0:512], in0=pk[:, 0:512],
                                        in1=kv_all[:, 0:512], op=OP.add)
                nc.vector.tensor_tensor(out=kv_all[:, 512:1024],
                                        in0=vts[:, 0:512],
                                        in1=kv_all[:, 512:1024], op=OP.add)
                continue
            kvsc = ACC4("kvsc", (128, KVP))
            for h in range(H):
                o = kvoff(h)
                hs = HSLOT[h]
                nc.tensor.matmul(kvsc[:, o:o + VW],
                                 pk[:, 128 * hs:128 * (hs + 1)],
                                 vts[:, VW * h:VW * (h + 1)],
                                 start=True, stop=True)
            for o0, w in ((0, 7 * VW), (512, 7 * VW), (1024, 2 * VW)):
                dst = kv_all[:, o0:o0 + w]
                if sc == 0:
                    nc.vector.tensor_copy(dst, kvsc[:, o0:o0 + w])
                else:
                    nc.vector.tensor_tensor(out=dst, in0=kvsc[:, o0:o0 + w],
                                            in1=dst, op=OP.add)

        if KPHASES < 4 or KP3 < 4:
            _finish_early()
            return
        # ==== Phase 4/5 overlap: pq for heads 0,1 hoisted above the ====
        # ==== collective so PE has work during the AllReduce        ====
        def _compute_pq(h):
            pq = SB("pq")
            qrow = SB("qrow", (64, T))
            nc.sync.dma_start(qrow[:], qkd[64 * h:64 * h + 64, :])
            hlrow = SB("hlrow", (2, T))
            nc.sync.dma_start(hlrow[:], qhld[h])
            for nt in range(NT):
                ppq = PP("ppq")
                nc.tensor.matmul(ppq[:], omd_t[0:64, :],
                                 qrow[:, 512 * nt:512 * (nt + 1)],
                                 start=True, stop=False)
                nc.tensor.matmul(ppq[:], allones[0:2, :],
                                 hlrow[:, 512 * nt:512 * (nt + 1)],
                                 start=False, stop=True)
                nc.scalar.activation(pq[:, 512 * nt:512 * (nt + 1)], ppq[:],
                                     AF.Exp)
            nc.vector.tensor_scalar_add(pq[:], pq[:], KERN_EPS)
            return pq
        pq_hoist = {h: _compute_pq(h) for h in (0, 1)}

        # ============ Phase 4: pair AllReduce of kv ============
        # bf16 collective: halves AllReduce payload (CC reduces in f32
        # internally; kvb is consumed as bf16 downstream anyway, so this only
        # adds one bf16 rounding before the pair-sum)
        cin_b = dram.tile([128, KVP], BF16, name="cin_b")
        cout_b = dram.tile([128, KVP], BF16, name="cout_b")
        kvh = rows.tile([128, KVP], BF16, name="kvh", tag="kvh")
        nc.vector.tensor_copy(kvh[:], kv_all[:])
        nc.gpsimd.dma_start(cin_b[:], kvh[:])
        nc.gpsimd.collective_compute(
            "AllReduce", OP.add,
            replica_groups=[[0, 1], [2, 3], [4, 5], [6, 7]],
            ins=[cin_b.opt()], outs=[cout_b.opt()])
        kvb = rows.tile([128, KVP], BF16, name="kvb", tag="kvb")
        nc.gpsimd.dma_start(kvb[:], cout_b[:])

        if KPHASES < 5:
            _finish_early()
            return
        # ==== Phase 5: pq + attn (token-major) + z + merge-transpose ====
        amt = []
        for hp in range(KC):
            atp = SB("atp")
            denp = rows.tile([128, 32], F32, name="denp", tag="denp")
            for h in (2 * hp, 2 * hp + 1):
                pq = pq_hoist[h] if h in pq_hoist else _compute_pq(h)
                ho = 64 * (h % 2)
                for sc in range(SC):
                    pat = PP("pat", (128, VW))
                    nc.tensor.matmul(pat[:], pq[:, 128 * sc:128 * (sc + 1)],
                                     kvb[:, kvoff(h):kvoff(h) + VW],
                                     start=True, stop=True)
                    if h % 2 == 0:
                        # Act has slack in phase 5; same Identity PSUM-drain
                        # pattern as the phase-9 output copies
                        nc.scalar.activation(
                            atp[:, 128 * sc + ho:128 * sc + ho + 64],
                            pat[:, 0:64], AF.Identity)
                    else:
                        nc.vector.tensor_copy(
                            atp[:, 128 * sc + ho:128 * sc + ho + 64],
                            pat[:, 0:64])
                    nc.vector.tensor_copy(
                        denp[:, 16 * (h % 2) + sc:16 * (h % 2) + sc + 1],
                        pat[:, 64:65])
            nc.vector.tensor_scalar_add(denp[:], denp[:], KERN_EPS)
            nc.vector.reciprocal(denp[:], denp[:])
            for h2 in range(2):
                for sc in range(SC):
                    sl = atp[:, 128 * sc + 64 * h2:128 * sc + 64 * h2 + 64]
                    nc.vector.tensor_scalar_mul(
                        sl, sl, denp[:, 16 * h2 + sc:16 * h2 + sc + 1])
            am = acts.tile([128, T], BF16, name=f"amt{hp}", tag=f"xt{hp}")
            for sc in range(SC):
                ptr = PP("ptr", (128, 128), BF16)
                nc.tensor.transpose(ptr[:], atp[:, 128 * sc:128 * (sc + 1)],
                                    identb[:])
                nc.vector.tensor_copy(am[:, 128 * sc:128 * (sc + 1)], ptr[:])
            amt.append(am)

        if KPHASES < 6:
            _finish_early()
            return
        # ============ Phase 6: Wo + residual -> r1d ============
        for oc in range(KC):
            wot = wstr.tile([128, KC * 128], BF16, name="wstrip", tag="wstrip")
            nc.scalar.dma_start(wot[:], io["wop"][oc])
            xtfs = SF("xtfs")
            nc.sync.dma_start(xtfs[:], xtfd[128 * oc:128 * (oc + 1), :])
            r1s = SF("r1s")
            for nt in range(NT):
                pwo = PP("pwo")
                for k in range(KC):
                    nc.tensor.matmul(pwo[:], wot[:, 128 * k:128 * (k + 1)],
                                     amt[k][:, 512 * nt:512 * (nt + 1)],
                                     start=(k == 0), stop=(k == KC - 1))
                nc.vector.scalar_tensor_tensor(
                    out=r1s[:, 512 * nt:512 * (nt + 1)], in0=pwo[:],
                    scalar=bo_t[:, oc:oc + 1],
                    in1=xtfs[:, 512 * nt:512 * (nt + 1)],
                    op0=OP.add, op1=OP.add)
            nc.sync.dma_start(r1d[128 * oc:128 * (oc + 1), :], r1s[:])

        # ============ Phase 7: LN1 -> out1 (bf16, resident) ============
        out1 = [acts.tile([128, T], BF16, name=f"out1_{oc}", tag=f"xt{oc}")
                for oc in range(KC)]
        _layer_norm(nc, PP, ACC4, r1d, out1, None,
                    g1_t, be1_t, allones, ones1f, identf, SB, SF, rows)

        if KPHASES < 8:
            _finish_early()
            return
        # ============ Phase 8: FFN -> r2d ============
        for nt in range(NT):
            h1 = rows.tile([128, FC * 512], BF16, name="h1", tag="h1")
            for fc in range(FC):
                if fc % 2 == 0:
                    w1t = wstr.tile([128, 2 * KC * 128], BF16, name="w1pair",
                                    tag="wstrip")
                    nc.scalar.dma_start(
                        w1t[:].rearrange("p (f c) -> p f c", f=2),
                        io["w1p"][fc:fc + 2].rearrange("f p c -> p f c"))
                w1o = 1024 * (fc % 2)
                ph = PP("ph")
                for k in range(KC):
                    nc.tensor.matmul(ph[:],
                                     w1t[:, w1o + 128 * k:w1o + 128 * (k + 1)],
                                     out1[k][:, 512 * nt:512 * (nt + 1)],
                                     start=(k == 0), stop=(k == KC - 1))
                eaer = SB("eaer", (128, 1024))
                nc.scalar.activation(eaer[:, 0:512], ph[:], AF.Exp,
                                     bias=b1_t[:, fc:fc + 1])
                nc.vector.tensor_scalar(out=eaer[:, 512:1024], in0=ph[:],
                                        scalar1=b1_t[:, fc:fc + 1], scalar2=0.0,
                                        op0=OP.add, op1=OP.max)
                nc.vector.scalar_tensor_tensor(
                    out=h1[:, 512 * fc:512 * (fc + 1)], in0=eaer[:, 0:512],
                    scalar=-1.0, in1=eaer[:, 512:1024], op0=OP.add, op1=OP.min)
            for half in range(2):
                pw2t = ACC4("pw2t")
                pw2 = [pw2t[:, 512 * j:512 * (j + 1)] for j in range(4)]
                for fc in range(FC):
                    if fc % 2 == 0:
                        w2t = wstr.tile([128, 1024], BF16, name="w2t", tag="w2t")
                        nc.scalar.dma_start(
                            w2t[:].rearrange("p (f c) -> p f c", f=2),
                            io["w2p"][fc:fc + 2][:, :, 512 * half:512 * (half + 1)]
                            .rearrange("f p c -> p f c"))
                    w2o = 512 * (fc % 2)
                    for j in range(4):
                        nc.tensor.matmul(pw2[j],
                                         w2t[:, w2o + 128 * j:w2o + 128 * (j + 1)],
                                         h1[:, 512 * fc:512 * (fc + 1)],
                                         start=(fc == 0), stop=(fc == FC - 1))
                for j in range(4):
                    oc = 4 * half + j
                    r2s = SF("r2s", (128, 512))
                    nc.vector.scalar_tensor_tensor(
                        out=r2s[:], in0=pw2[j], scalar=b2_t[:, oc:oc + 1],
                        in1=out1[oc][:, 512 * nt:512 * (nt + 1)],
                        op0=OP.add, op1=OP.add)
                    nc.scalar.dma_start(
                        r2d[128 * oc:128 * (oc + 1), 512 * nt:512 * (nt + 1)],
                        r2s[:])

        # ============ Phase 9: LN2 -> out (token-major f16) ============
        _layer_norm(nc, PP, ACC4, r2d, None, io["out"],
                    g2_t, be2_t, allones, ones1f, identf, SB, SF, rows)


def _layer_norm(nc, PP, ACC4, rind, out_bf, out_dram,
                g_t, be_t, allones, ones1f, identf, SB, SF, rows):
    """Feature-major LN over the DRAM-resident [D, T] tensor `rind`.

    out_bf: list of 8 resident bf16 [128, T] tiles, or None -> transpose to
    token-major f16 and write to `out_dram` ([T, D] DRAM tensor).
    """
    tagp = "1" if out_bf is not None else "2"
    rowA = rows.tile([1, T], F32, name=f"rowA{tagp}", tag="rowA")
    rowB = rows.tile([1, T], F32, name=f"rowB{tagp}", tag="rowB")
    # --- stats in one pass: sum -> psum row 0, sumsq -> psum row 32
    # (disjoint partitions of one ACC4 bank group; inputs both base-0)
    pst = ACC4("pst", (33, T))
    for oc in range(KC):
        rsrc = SF("lnsrc")
        nc.sync.dma_start(rsrc[:], rind[128 * oc:128 * (oc + 1), :])
        rb = SB("lnrb")
        nc.vector.tensor_copy(rb[:], rsrc[:])
        rs = SB("lnrs")
        nc.vector.tensor_tensor(out=rs[:], in0=rb[:], in1=rb[:], op=OP.mult)
        for nt in range(NT):
            nc.tensor.matmul(pst[0:1, 512 * nt:512 * (nt + 1)],
                             allones[:, 0:1], rb[:, 512 * nt:512 * (nt + 1)],
                             start=(oc == 0), stop=(oc == KC - 1))
            nc.tensor.matmul(pst[32:33, 512 * nt:512 * (nt + 1)],
                             allones[:, 0:1], rs[:, 512 * nt:512 * (nt + 1)],
                             start=(oc == 0), stop=(oc == KC - 1))
    nc.vector.tensor_scalar_mul(rowA[:], pst[0:1, :], 1.0 / D)
    nc.vector.tensor_scalar_mul(rowB[:], pst[32:33, :], 1.0 / D)
    # --- rows: rowA=mu, rowB=ex2 -> rowB=rstd, rowA=-mu*rstd
    musq = ACC4("pst2", (1, T))
    nc.vector.tensor_tensor(out=musq[:], in0=rowA[:], in1=rowA[:], op=OP.mult)
    nc.vector.scalar_tensor_tensor(out=rowB[:], in0=rowB[:], scalar=LN_EPS,
                                   in1=musq[:], op0=OP.add, op1=OP.subtract)
    nc.vector.reciprocal(musq[:], rowB[:])
    nc.scalar.activation(rowB[:], musq[:], AF.Sqrt)          # rstd
    nc.vector.scalar_tensor_tensor(out=rowA[:], in0=rowA[:], scalar=-1.0,
                                   in1=rowB[:], op0=OP.mult, op1=OP.mult)
    # --- broadcast rstd / (-mu*rstd) to [128, T] via K=1 f32 matmuls
    rstd_bc = rows.tile([128, T], F32, name=f"rstd_bc{tagp}", tag="rstd_bc")
    bneg_bc = rows.tile([128, T], F32, name=f"bneg_bc{tagp}", tag="bneg_bc")
    for nt in range(NT):
        pa = PP("pa")
        nc.tensor.matmul(pa[:], ones1f[:], rowB[0:1, 512 * nt:512 * (nt + 1)],
                         start=True, stop=True)
        nc.vector.tensor_copy(rstd_bc[:, 512 * nt:512 * (nt + 1)], pa[:])
        pb = PP("pb")
        nc.tensor.matmul(pb[:], ones1f[:], rowA[0:1, 512 * nt:512 * (nt + 1)],
                         start=True, stop=True)
        nc.vector.tensor_copy(bneg_bc[:, 512 * nt:512 * (nt + 1)], pb[:])
    # --- apply: out = (r*g*rstd) + (bneg*g) + beta
    for oc in range(KC):
        rsrc = SF("lnap")
        nc.sync.dma_start(rsrc[:], rind[128 * oc:128 * (oc + 1), :])
        nc.vector.scalar_tensor_tensor(out=rsrc[:], in0=rsrc[:],
                                       scalar=g_t[:, oc:oc + 1], in1=rstd_bc[:],
                                       op0=OP.mult, op1=OP.mult)
        nc.vector.scalar_tensor_tensor(out=rsrc[:], in0=bneg_bc[:],
                                       scalar=g_t[:, oc:oc + 1], in1=rsrc[:],
                                       op0=OP.mult, op1=OP.add)
        if out_bf is not None:
            nc.scalar.activation(out_bf[oc][:], rsrc[:], AF.Identity,
                                 bias=be_t[:, oc:oc + 1])
        else:
            ost = SF("ost")
            nc.scalar.activation(ost[:], rsrc[:], AF.Identity,
                                 bias=be_t[:, oc:oc + 1])
            o16 = SB("o16", (128, T), F16)
            for sc in range(SC):
                pto = PP("pto", (128, 128))
                nc.tensor.transpose(pto[:], ost[:, 128 * sc:128 * (sc + 1)],
                                    identf[:])
                nc.scalar.activation(o16[:, 128 * sc:128 * (sc + 1)], pto[:],
                                     AF.Identity)
            nc.sync.dma_start(
                out_dram[:, 128 * oc:128 * (oc + 1)].rearrange(
                    "(s p) c -> p s c", s=SC),
                o16[:].rearrange("p (s c) -> p s c", s=SC))


# ======================= host side =======================

def _prep_common(inputs):
    scale = float(DH) ** -0.25
    f = lambda a: np.ascontiguousarray(np.asarray(a, np.float32))
    bf = lambda a: np.ascontiguousarray(np.asarray(a).astype(ml_dtypes.bfloat16))

    Wq, Wk, Wv, Wo = f(inputs["Wq"]), f(inputs["Wk"]), f(inputs["Wv"]), f(inputs["Wo"])
    bq, bk, bv, bo = f(inputs["bq"]), f(inputs["bk"]), f(inputs["bv"]), f(inputs["bo"])
    W1, W2, b1, b2 = f(inputs["W1"]), f(inputs["W2"]), f(inputs["b1"]), f(inputs["b2"])
    omega = f(inputs["omega"])

    wqk = np.concatenate([Wq * scale, Wk * scale], axis=1)          # [D, 2D]
    wqkp = wqk.reshape(KC, 128, 16, 128).transpose(2, 1, 0, 3).reshape(16, 128, KC * 128)
    bqk = np.concatenate([bq * scale, bk * scale]).reshape(16, 128).T.copy()

    wv_aug = np.zeros((D, H * VW), np.float32)
    bva = np.zeros((1, H * VW), np.float32)
    for h in range(H):
        wv_aug[:, VW * h:VW * h + DH] = Wv[:, DH * h:DH * (h + 1)]
        bva[0, VW * h:VW * h + DH] = bv[DH * h:DH * (h + 1)]
        bva[0, VW * h + DH] = 1.0
    # wvpj[j][p, VJ*k + c] = wv_aug[128k + p, VJ*j + c]
    wvpj = wv_aug.reshape(KC, 128, 4, VJ).transpose(2, 1, 0, 3).reshape(4, 128, KC * VJ)

    omt = omega.T.copy()                                             # [DH, M]
    omd = np.concatenate([omt, omt], axis=0)                         # [128, M]

    # eh_t[:, 16k:16(k+1)]: chunk k holds heads 2k (rows 0:64), 2k+1 (64:128)
    ehalf = np.zeros((128, KC * H), np.float32)
    for k in range(KC):
        ehalf[0:64, 16 * k + 2 * k] = -0.5
        ehalf[64:128, 16 * k + 2 * k + 1] = -0.5

    wop = Wo.reshape(KC, 128, KC, 128).transpose(2, 1, 0, 3).reshape(KC, 128, KC * 128)
    w1p = W1.reshape(KC, 128, FC, 128).transpose(2, 1, 0, 3).reshape(FC, 128, KC * 128)
    w2p = W2.reshape(FC, 128, KC * 128)

    col = lambda v: np.asarray(v, np.float32).reshape(KC, 128).T.copy()
    colf = lambda v: np.asarray(v, np.float32).reshape(FC, 128).T.copy()

    return {
        "wqkp": bf(wqkp), "wvpj": bf(wvpj), "bqk": bqk, "bva": bf(bva),
        "omd": bf(omd), "ehalf": bf(ehalf),
        "wop": bf(wop), "bo": col(bo),
        "w1p": bf(w1p), "b1": colf(b1),
        "w2p": bf(w2p), "b2": col(b2),
        "g1": col(inputs["g1"]), "be1": col(inputs["beta1"]),
        "g2": col(inputs["g2"]), "be2": col(inputs["beta2"]),
    }


def _get_runner():
    """Build (once) a jitted SPMD executor: x sharded over the token axis,
    weights replicated (P()), so weights transfer to one core and replicate
    on-device."""
    if "runner" in _CACHE:
        return _CACHE["runner"]
    import jax
    import jax.numpy as jnp
    from jax.sharding import Mesh, PartitionSpec, NamedSharding
    from jax.experimental.shard_map import shard_map
    from concourse import bass2jax

    if "nc" not in _CACHE:
        _CACHE["nc"] = _build()
    nc = _CACHE["nc"]
    bass2jax.install_neuronx_cc_hook()

    partition_name = nc.partition_id_tensor.name if nc.partition_id_tensor else None
    in_names, out_names, out_avals = [], [], []
    for alloc in nc.m.functions[0].allocations:
        if not isinstance(alloc, mybir.MemoryLocationSet):
            continue
        name = alloc.memorylocations[0].name
        if alloc.kind == "ExternalInput":
            if name != partition_name:
                in_names.append(name)
        elif alloc.kind == "ExternalOutput":
            shape = tuple(alloc.tensor_shape)
            out_avals.append(jax.core.ShapedArray(shape, mybir.dt.np(alloc.dtype)))
            out_names.append(name)
    n_params = len(in_names)
    all_names = tuple(in_names) + tuple(out_names) + (
        (partition_name,) if partition_name else ())

    def _body(*args):
        operands = list(args)
        if partition_name is not None:
            operands.append(bass2jax.partition_id_tensor())
        outs = bass2jax._bass_exec_p.bind(
            *operands,
            out_avals=tuple(out_avals),
            in_names=all_names,
            out_names=tuple(out_names),
            lowering_input_output_aliases=(),
            sim_require_finite=True,
            sim_require_nnan=True,
            nc=nc,
        )
        return tuple(outs)

    devices = jax.devices()[:NCORES]
    mesh = Mesh(np.asarray(devices), ("core",))
    n_outs = len(out_names)
    in_specs = tuple(PartitionSpec("core") if n == "xin" else PartitionSpec()
                     for n in in_names) + (PartitionSpec("core"),) * n_outs
    sharded = jax.jit(
        shard_map(_body, mesh=mesh,
                  in_specs=in_specs,
                  out_specs=(PartitionSpec("core"),) * n_outs,
                  check_rep=False),
        donate_argnums=tuple(range(n_params, n_params + n_outs)),
        keep_unused=True)
    shard = NamedSharding(mesh, PartitionSpec("core"))
    repl = NamedSharding(mesh, PartitionSpec())
    zero_makers = [
        jax.jit(lambda av=av: jnp.zeros((NCORES * av.shape[0],) + av.shape[1:],
                                        av.dtype),
                out_shardings=shard)
        for av in out_avals]
    _CACHE["runner"] = (sharded, in_names, out_names, out_avals, shard, repl,
                        devices, zero_makers)
    return _CACHE["runner"]


def _crc(a):
    a = np.ascontiguousarray(a)
    return zlib.crc32(memoryview(a).cast("B"))


def _stage_weights(inputs):
    """Prep + stage the (replicated) weight tensors; cached across calls."""
    import jax
    key = tuple((k,) + tuple(np.shape(inputs[k])) + (_crc(inputs[k]),)
                for k in RAW_WEIGHT_KEYS)
    if _CACHE.get("wkey") == key:
        return _CACHE["wdev"]
    sharded, in_names, out_names, out_avals, shard, repl, devices, zm = _get_runner()
    common = _prep_common(inputs)
    # one host->device transfer per tensor (to core 0), then on-device
    # replication via reshard; both issued async, blocked once.
    d0 = {n: jax.device_put(common[n], devices[0]) for n in WEIGHT_NAMES}
    wdev = {n: jax.device_put(d0[n], repl) for n in WEIGHT_NAMES}
    jax.block_until_ready(list(wdev.values()))
    _CACHE["wkey"] = key
    _CACHE["wdev"] = wdev
    return wdev


def _stage_x(x):
    """Stage x in its natural [B*S, D] layout, sharded over tokens; cached."""
    import jax
    x = np.ascontiguousarray(np.asarray(x, np.float32))
    key = (x.shape, _crc(x))
    if _CACHE.get("xkey") == key:
        return _CACHE["xdev"]
    sharded, in_names, out_names, out_avals, shard, repl, devices, zm = _get_runner()
    xdev = jax.device_put(x.reshape(B * S, D), shard)
    xdev.block_until_ready()
    _CACHE["xkey"] = key
    _CACHE["xdev"] = xdev
    return xdev


def _run_staged(staged):
    sharded, in_names, out_names, out_avals, shard, repl, devices, zero_makers = _get_runner()
    zeros = [zm() for zm in zero_makers]
    outs = sharded(*staged, *zeros)
    return {name: outs[i] for i, name in enumerate(out_names)}


def _staged_list(inputs):
    wdev = _stage_weights(inputs)
    xdev = _stage_x(inputs["x"])
    sharded, in_names, out_names, out_avals, shard, repl, devices, zm = _get_runner()
    return [xdev if n == "xin" else wdev[n] for n in in_names]


def kernel(**inputs):
    staged = _staged_list(inputs)
    outs = _run_staged(staged)
    o = np.asarray(outs["out"])            # [B*S, D] f16 token-major
    return o.astype(np.float32).reshape(B, S, D)


def bench_exec_ns(inputs, iters=40):
    """Estimate per-execution device time (ns).

    The axon tunnel adds ~80 ms of fixed dispatch latency per call and gives
    no NTFF profile hook, so a single-call wall time says nothing about the
    hardware. Instead we time `iters` back-to-back executions and subtract
    the same count of trivial-op dispatches (identical dispatch path), i.e.
    the marginal cost of executing this NEFF on the cores.
    """
    import time as _time
    import jax
    staged = _staged_list(inputs)
    r = _run_staged(staged)
    jax.block_until_ready(list(r.values()))

    if "triv" not in _CACHE:
        _CACHE["triv"] = jax.jit(lambda a: a + 1.0)
    triv = _CACHE["triv"]
    sharded, in_names, *_rest = _get_runner()
    small = staged[in_names.index("bo")]
    jax.block_until_ready(triv(small))

    def loop(fn, n):
        best = float("inf")
        for _ in range(3):
            t0 = _time.perf_counter()
            rs = [fn() for _ in range(n)]
            for rr in rs:
                jax.block_until_ready(rr)
            best = min(best, _time.perf_counter() - t0)
        return best

    def run_once():
        rr = _run_staged(staged)
        return list(rr.values())

    tk = loop(run_once, iters)
    tf = loop(lambda: triv(small), iters)
    est = (tk - tf) / iters
    if est <= 0:
        est = 1e-9
    return est * 1e9


if __name__ == "__main__":
    nc = _build()
    print("build ok")
